# revision 22
# baseline (speedup 1.0000x reference)
"""EqualizedConv2dModulated Trainium2 kernel.

Math (per sample b):
    out[b,o] = (1/sigma[b,o]) * conv2d_SAME(s[b,:]*x[b], weight)[o]
    sigma[b,o] = sqrt( sum_i s[b,i]^2 * (sum_tap weight[o,i,tap]^2) + EPS )

This is algebraically identical to the reference (modulate weights, L2
demodulate, grouped conv) but turns the grouped conv into a standard conv
with shared weights: fold s into x, fold 1/sigma into the output.

Sharding: data-parallel over batch. 8 cores x 2 samples each, full weight
replica per core, no collectives.

Per-core device program:
  - weight is DMA'd o-major (contiguous), transposed to i-major [i, tap, o]
    on the tensor engine (128x128 PE transposes through PSUM),
  - w2[i,o] = sum_tap w^2 computed from the o-major chunks (square + reduce
    + PE transpose), sigma^2 via tiny matmuls against s^2,
  - x is modulated by s and written into a zero-padded [128, 34, 34] SBUF
    image per (sample, i-tile),
  - conv = 36 accumulating matmuls (4 i-tiles x 9 taps) per PSUM tile of
    [128 o, 512 px], eviction fused with the 1/sigma scale on the DVE.
"""

import os
import sys

sys.path.insert(0, "/opt/trn_rl_repo")

import numpy as np

import concourse.bass as bass
import concourse.mybir as mybir
from concourse.bass_utils import run_bass_kernel_spmd
from concourse.masks import make_identity
from concourse.tile import TileContext

N_CORES = 8
B, I, O, H, W = 16, 512, 512, 32, 32
BL = B // N_CORES  # samples per core
NT = I // 128  # i tiles
OT = O // 128  # o tiles
HB = 2  # h blocks of 16 rows (16*32 = 512 px per matmul)
EPS = 1e-8
F32 = mybir.dt.float32

# Matmul operand dtype for the conv. float32 is exact; float32r / bfloat16
# run the PE at 4x the fp32 rate.
CONV_DT = {
    "f32": mybir.dt.float32,
    "f32r": mybir.dt.float32r,
    "bf16": mybir.dt.bfloat16,
}[os.environ.get("CONV_DT", "f32")]


def _emit(nc, x_ext, s_ext, w_ext, out_ext, tc):
    # Engine/wait discipline (walrus sync-wait capacities: self-loading
    # fp32/fp32r matmul = 1, DMA = 2, ACT/DVE/Pool = many):
    #  - every tile a PE instruction reads is produced by ACT (or observed
    #    earlier), so PE instructions carry at most one ACT-sem wait;
    #  - per-chunk "dummy" transposes absorb the DMA wait before the real
    #    transposes touch a freshly-DMA'd chunk (f32 path);
    #  - chunk staging uses bufs=8: with 16 back-to-back chunk DMAs over the
    #    8 round-robin DMAHW sem lanes, the slot-WAW wait lands on the same
    #    lane sem as the FIFO-order wait and merges, keeping DMAs at <=2.
    fkind = CONV_DT != mybir.dt.bfloat16  # 4-byte tile path (f32 / f32r)
    # PE-operand tiles carry the conv dtype; their ACT producers emit
    # properly ROUNDED values (the BIR verifier requires fp32r matmul
    # operands to be produced as fp32r, so no bitcasting).
    TDT = CONV_DT

    with (
        tc.tile_pool(name="const", bufs=1) as constp,
        tc.tile_pool(name="wstage", bufs=4) as wstage,
        tc.tile_pool(name="wt", bufs=1) as wtp,
        tc.tile_pool(name="xp", bufs=1) as xpp,
        tc.tile_pool(name="sq", bufs=2) as sqp,
        tc.tile_pool(name="outp", bufs=8) as outp,
        tc.tile_pool(name="ps_t", bufs=2, space="PSUM") as ps_tp,
        tc.tile_pool(name="ps_sig", bufs=1, space="PSUM") as ps_sigp,
        tc.tile_pool(name="ps_conv", bufs=4, space="PSUM") as ps_convp,
    ):
        # --- identity bootstrap ------------------------------------------
        id_gp = constp.tile([128, 128], F32, tag="id_gp")
        make_identity(nc, id_gp)
        epsb = constp.tile([128, 1], F32, tag="epsb")
        nc.gpsimd.memset(epsb, EPS)
        ps_id = ps_tp.tile([128, 128], F32, name="ps_id", tag="ps_id", bufs=1)
        nc.tensor.transpose(ps_id, id_gp, id_gp)
        identity = constp.tile([128, 128], F32, tag="identity")
        nc.scalar.copy(identity, ps_id)
        # re-absorb ps_id's WAR release (ACT) so later dummies only ever
        # wait on their chunk's DMA lane
        nc.tensor.transpose(ps_id, id_gp, id_gp)
        # ACT-side absorber for the eps constant (Pool-produced)
        epsb_act = constp.tile([128, 1], F32, tag="epsb_act")
        nc.scalar.copy(epsb_act, epsb)

        # --- s tiles: [i_p, b] per i-tile, squares on DVE ----------------
        s_t, s2_t = [], []
        for it in range(NT):
            st = constp.tile([128, BL], F32, name=f"s_t{it}", tag=f"s_t{it}")
            nc.sync.dma_start(
                out=st, in_=s_ext[:, it * 128 : (it + 1) * 128].rearrange("b i -> i b")
            )
            s2 = constp.tile([128, BL], F32, name=f"s2_t{it}", tag=f"s2_t{it}")
            nc.vector.tensor_mul(s2, st, st)
            # ACT-side absorber so modulates don't add a second (DMA) wait
            sa = constp.tile([128, BL], F32, name=f"s_a{it}", tag=f"s_a{it}")
            nc.scalar.copy(sa, st)
            s_t.append(sa)
            s2_t.append(s2)

        # --- weight transpose: [o,i,tap] -> w_t[it][i_p, tap, o] ---------
        w_t = [
            wtp.tile([128, 9, O], TDT, name=f"w_t{it}", tag=f"w_t{it}")
            for it in range(NT)
        ]
        for ot in range(OT):
            for it in range(NT):
                chunk = wstage.tile([128, 128, 3, 3], F32, name="chunk", tag="chunk")
                nc.sync.dma_start(
                    out=chunk,
                    in_=w_ext[
                        ot * 128 : (ot + 1) * 128, it * 128 : (it + 1) * 128, :, :
                    ],
                )
                # dummy transpose: its only wait is the chunk's DMA lane;
                # after it the PE has observed that lane for the real ones
                nc.tensor.transpose(ps_id, chunk[:, :, 0, 0], identity)
                for g in range(3):  # 3 taps per PSUM tile, 1 packed copy out
                    pst = ps_tp.tile([128, 3, 128], F32, name="pst", tag="pst")
                    for j in range(3):
                        kh, kw = divmod(3 * g + j, 3)
                        nc.tensor.transpose(
                            pst[:, j, :], chunk[:, :, kh, kw], identity
                        )
                    nc.scalar.copy(
                        w_t[it][:, 3 * g : 3 * g + 3, ot * 128 : (ot + 1) * 128], pst
                    )

        # --- w2[it][i_p, o] = sum_tap w^2 (DVE, from transposed weights) -
        w2 = [
            wtp.tile([128, O], F32, name=f"w2_{it}", tag=f"w2_{it}")
            for it in range(NT)
        ]
        for it in range(NT):
            for ot in range(OT):
                osl = slice(ot * 128, (ot + 1) * 128)
                sqs = sqp.tile([128, 9, 128], F32, name="sqs", tag="sqs")
                nc.vector.tensor_mul(sqs, w_t[it][:, :, osl], w_t[it][:, :, osl])
                nc.vector.tensor_reduce(
                    w2[it][:, osl],
                    sqs.rearrange("p t o -> p o t"),
                    axis=mybir.AxisListType.X,
                    op=mybir.AluOpType.add,
                )

        # --- sigma: rinv_all[o_p, ot*BL+b] = 1/sqrt(sigma2 + eps) --------
        ps_all = ps_sigp.tile([128, OT * BL], F32, name="ps_all", tag="ps_all")
        for ot in range(OT):
            for it in range(NT):
                nc.tensor.matmul(
                    ps_all[:, ot * BL : (ot + 1) * BL],
                    lhsT=w2[it][:, ot * 128 : (ot + 1) * 128],
                    rhs=s2_t[it],
                    start=(it == 0),
                    stop=(it == NT - 1),
                )
        sig_all = constp.tile([128, OT * BL], F32, tag="sig_all")
        nc.scalar.activation(
            sig_all, ps_all, func=mybir.ActivationFunctionType.Sqrt, bias=epsb_act
        )
        rinv_dve = constp.tile([128, OT * BL], F32, tag="rinv_dve")
        nc.vector.reciprocal(rinv_dve, sig_all)
        # ACT-side absorber so evictions wait only on the PE (psum) sem
        rinv_all = constp.tile([128, OT * BL], F32, tag="rinv_all")
        nc.scalar.copy(rinv_all, rinv_dve)

        # --- x: modulate by s into zero-padded [i_p, 34, 34] -------------
        xpad = [[None] * NT for _ in range(BL)]
        for b in range(BL):
            for it in range(NT):
                xp = xpp.tile(
                    [128, H + 2, W + 2], TDT, name=f"xpad_{b}_{it}",
                    tag=f"xpad_{b}_{it}",
                )
                nc.scalar.activation(
                    xp,
                    epsb_act[:, 0:1].to_broadcast((128, H + 2, W + 2)),
                    func=mybir.ActivationFunctionType.Copy,
                    scale=0.0,
                )
                s_ap = s_t[it][:, b : b + 1]
                if fkind:
                    # DMA straight into the padded interior; modulate in
                    # place on ACT (conv then sees only ACT as writer)
                    nc.sync.dma_start(
                        out=xp[:, 1 : H + 1, 1 : W + 1],
                        in_=x_ext[b, it * 128 : (it + 1) * 128, :, :],
                    )
                    nc.scalar.mul(
                        xp[:, 1 : H + 1, 1 : W + 1], xp[:, 1 : H + 1, 1 : W + 1], s_ap
                    )
                else:
                    xf = constp.tile(
                        [128, H, W], F32, name=f"xf_{b}_{it}", tag=f"xf_{b}_{it}"
                    )
                    nc.sync.dma_start(
                        out=xf, in_=x_ext[b, it * 128 : (it + 1) * 128, :, :]
                    )
                    nc.scalar.mul(xp[:, 1 : H + 1, 1 : W + 1], xf, s_ap)
                xpad[b][it] = xp

        # --- conv: 36 accumulating matmuls per [o_p, 512 px] PSUM tile ---
        obs = []
        for b in range(BL):
            for ot in range(OT):
                for hb in range(HB):
                    ps = ps_convp.tile([128, 512], F32, name="psc", tag="psc")
                    step = 0
                    for it in range(NT):
                        for tap in range(9):
                            kh, kw = divmod(tap, 3)
                            rhs = xpad[b][it][
                                :, hb * 16 + kh : hb * 16 + kh + 16, kw : kw + 32
                            ]
                            nc.tensor.matmul(
                                ps,
                                lhsT=w_t[it][:, tap, ot * 128 : (ot + 1) * 128],
                                rhs=rhs,
                                start=(step == 0),
                                stop=(step == NT * 9 - 1),
                            )
                            step += 1
                    gi = (b * OT + ot) * HB + hb
                    ob = outp.tile(
                        [128, 512], F32, name=f"ob{gi}", tag=f"ob{gi}", bufs=1
                    )
                    # eviction fused with the 1/sigma scale, on ACT so the
                    # PSUM-slot release is an ACT sem too
                    nc.scalar.mul(ob, ps, rinv_all[:, ot * BL + b : ot * BL + b + 1])
                    nc.sync.dma_start(
                        out=out_ext[
                            b, ot * 128 : (ot + 1) * 128, hb * 16 : hb * 16 + 16, :
                        ].rearrange("o h w -> o (h w)"),
                        in_=ob,
                    )
                    obs.append(ob)

        # sync ladder: one ACT write per ob tile (WAR on its out-store) walks
        # every out-DMA completion into the ACT clock, so the kernel-end
        # drain's 12 proc waits all become implied and strip down to one.
        for ob in obs:
            nc.scalar.memzero(ob[:, 0:1])


def _strip_implied_waits(nc):
    """Drop sem waits that are transitively implied by the instruction's
    remaining waits plus its engine/ring program order. Tile's wait pass is
    per-proc minimal but not transitively minimal, and walrus caps
    self-loading matmuls and DIRECT2D DMAs at ONE sync wait.

    Clock semantics (valid because per-lane updates stay in order: a lane
    wait is only stripped when the kept waits already imply the previous
    same-lane update fired): "sem >= v" implies the prefix of updates (in
    scheduled order) whose cumulative value first reaches v has completed,
    carrying the join of those updaters' completion clocks.
    """
    import bass_rust
    import concourse.mybir as mybir
    from collections import defaultdict

    insts = [
        inst
        for f in nc.m.functions
        for blk in f.blocks
        for inst in blk.instructions
        if getattr(inst, "sync_info", None) is not None
    ]

    sem_hist = defaultdict(list)  # sem id -> [(cum_after_update, completion_clock)]
    sem_cum = defaultdict(int)
    eng_clock = defaultdict(dict)  # engine -> completion clock of last inst
    ring_clock = defaultdict(dict)  # issuing engine -> start clock of last DMA

    EXEMPT = {"InstEventSemaphore", "InstMemset"}
    DRAIN_LIMIT = 1

    def join(dst, srcs):
        for s in srcs:
            for k, v in s.items():
                if dst.get(k, 0) < v:
                    dst[k] = v
        return dst

    def wait_clock(sem_id, val):
        c = {sem_id: val}
        for cum, cclock in sem_hist[sem_id]:
            if cum <= val:
                join(c, [cclock])
            else:
                break
        return c

    def covers(clock, sem_id, val):
        return clock.get(sem_id, 0) >= val

    n_stripped = 0
    for inst in insts:
        si = inst.sync_info
        kind = type(inst).__name__
        is_dma = kind == "InstDMACopy"
        # Lane-order waits on the final DRAM stores are droppable: nothing
        # waits on the out-lane sems at intermediate values except
        # instructions that are transitive dependencies of every out store
        # (all input DMAs feed the conv), and the kernel-end drain waits on
        # the order-independent cumulative total.
        is_out_store = is_dma and any(
            getattr(o, "memref", "") == "out" for o in inst.outs
        )
        eng = inst.engine
        base = dict(ring_clock[eng]) if is_dma else dict(eng_clock[eng])
        waits = [
            w
            for w in si.on_wait
            if w.sync_type == "semaphore" and w.wait_mode == "sem-ge-imm"
        ]
        other = [w for w in si.on_wait if w not in waits]
        limit = None if kind in EXEMPT else 1
        if limit is not None and len(si.on_wait) > limit:
            # greedily drop implied waits
            kept = list(waits)
            changed = True
            while changed and len(kept) + len(other) > limit:
                changed = False
                own_sems = {u.id for u in si.on_update if u.sync_type == "semaphore"}
                for w in list(kept):
                    rest = [x for x in kept if x is not w]
                    c = dict(base)
                    join(c, [wait_clock(x.id, x.wait_value) for x in rest])
                    if (is_out_store and w.id in own_sems) or covers(
                        c, w.id, w.wait_value
                    ):
                        kept.remove(w)
                        n_stripped += 1
                        changed = True
                        break
            if len(kept) + len(other) > limit and not other:
                # escalate: replace all waits with one later wait on a single
                # sem whose prefix-clock covers every dropped wait (waiting
                # longer is safe; producers never depend on this instruction)
                for w in kept:
                    acc = dict(base)
                    hist = sem_hist[w.id]
                    pick = None
                    for cum, cclock in hist:
                        join(acc, [cclock])
                        acc[w.id] = max(acc.get(w.id, 0), cum)
                        if cum >= w.wait_value and all(
                            covers(acc, x.id, x.wait_value)
                            for x in kept
                            if x is not w
                        ):
                            pick = cum
                            break
                    if pick is not None:
                        nw = bass_rust.SyncWait(
                            sync_type=w.sync_type,
                            id=w.id,
                            ant_name=w.ant_name,
                            wait_mode=w.wait_mode,
                            wait_value=pick,
                            wait_reg=None,
                        )
                        kept = [nw]
                        n_stripped += 1
                        break
            if len(kept) != len(waits):
                inst.sync_info = bass_rust.SyncInfo(
                    on_wait=other + kept, on_update=list(si.on_update)
                )
                si = inst.sync_info
                waits = kept
        # advance clocks
        start = dict(base)
        join(start, [wait_clock(w.id, w.wait_value) for w in waits])
        compl = dict(start)
        for u in si.on_update:
            if u.sync_type == "semaphore":
                sem_cum[u.id] += u.update_value
                compl[u.id] = max(compl.get(u.id, 0), sem_cum[u.id])
        if is_dma:
            ring_clock[eng] = start
        else:
            eng_clock[eng] = compl
        for u in si.on_update:
            if u.sync_type == "semaphore":
                sem_hist[u.id].append((sem_cum[u.id], compl))
    return n_stripped


def _validate_waits(nc):
    """Pre-compile check of walrus sync-wait capacities."""
    bad = []
    for f in nc.m.functions:
        for blk in f.blocks:
            for inst in blk.instructions:
                si = getattr(inst, "sync_info", None)
                if si is None:
                    continue
                n = len(si.on_wait)
                kind = type(inst).__name__
                limit = (
                    99
                    if kind in ("InstEventSemaphore", "InstMemset")
                    else 1
                )
                if n > limit:
                    bad.append((inst.name, kind, n, si.on_wait))
    if bad:
        for name, kind, n, waits in bad[:8]:
            print(f"WAIT-LIMIT {name} {kind}: {n} waits: "
                  f"{[w.ant_name for w in waits]}")
        raise RuntimeError(f"{len(bad)} instructions exceed sync-wait limits")


_NC_CACHE = None


def _build_nc():
    global _NC_CACHE
    if _NC_CACHE is not None:
        return _NC_CACHE
    nc = bass.Bass(target_bir_lowering=False)
    xdt = CONV_DT if CONV_DT != mybir.dt.bfloat16 else F32
    x_ext = nc.declare_dram_parameter("x", [BL, I, H, W], xdt, isOutput=False)
    s_ext = nc.declare_dram_parameter("s", [BL, I], F32, isOutput=False)
    w_ext = nc.declare_dram_parameter("weight", [O, I, 3, 3], F32, isOutput=False)
    out_ext = nc.declare_dram_parameter("out", [BL, O, H, W], F32, isOutput=True)
    with TileContext(nc) as tc:
        _emit(nc, x_ext, s_ext, w_ext, out_ext, tc)
    _strip_implied_waits(nc)
    _validate_waits(nc)
    _NC_CACHE = nc
    return nc


LAST_RESULTS = None  # BassKernelResults from the most recent kernel() call


def kernel(x, s, weight):
    global LAST_RESULTS
    x = np.ascontiguousarray(np.asarray(x, dtype=np.float32))
    s = np.ascontiguousarray(np.asarray(s, dtype=np.float32))
    weight = np.ascontiguousarray(np.asarray(weight, dtype=np.float32))
    assert x.shape == (B, I, H, W) and s.shape == (B, I)
    assert weight.shape == (O, I, 3, 3)

    nc = _build_nc()
    in_maps = [
        {
            "x": x[c * BL : (c + 1) * BL],
            "s": s[c * BL : (c + 1) * BL],
            "weight": weight,
        }
        for c in range(N_CORES)
    ]
    res = run_bass_kernel_spmd(nc, in_maps, list(range(N_CORES)))
    LAST_RESULTS = res
    out = np.concatenate([res.results[c]["out"] for c in range(N_CORES)], axis=0)
    return out.astype(np.float32)
